# revision 65
# baseline (speedup 1.0000x reference)
"""Multi-head attention (B=2, S=2048, D=1024, H=16, dk=64) on 8 Trainium2
NeuronCores via Bass/Tile.

Sharding: core c handles batch b = c//4 and head-group g = c%4 (4 heads,
256 qkv columns).  Each core computes its QKV projection slices, 4 heads of
attention, and a partial output projection against its 256-row slice of Wo.
The host sums the 4 partial outputs per batch (row-sharded Wo => partial
sums) and folds in the biases bo and bv@Wo (softmax rows sum to 1, so the
V-bias contributes exactly bv@Wo per token).

v3 design notes (vs v2's f32r):
- Everything bf16: fp32 feeds the PE at half the bf16 streaming rate
  (measured 1.2 rows/ns vs 2.4), so bf16 operands double matmul throughput.
  PSUM accumulation stays f32.  Measured numeric impact ~2.3e-3 rel_l2.
- Weights are cast to bf16 host-side -> straight DMA, no on-chip casts.
- x is DMA-transposed once (bf16, no hi/lo split).
- The exp on the ACT engine (~1.1us per [128,2,512] chunk) paces the
  attention phase; the PE (~182us busy) is the global bottleneck.  Only
  K(0)/V(0)/Q(0) of the p=0 column tile run as a lead-in; every other
  projection column-tile, V unit, and output-projection unit is emitted as
  "filler" PE work inside the attention kc loop (demand-driven via
  ensure(), which force-emits producer groups before their consumers so
  dependencies are recorded), hiding under the exp stream instead of
  serializing before/after it.  Scores PSUM is two parity tiles so the
  tile-granular WAR dep doesn't collapse the sc->exp pipeline to 1-deep,
  and each block's first two score pairs are emitted during the previous
  block's last two slots to cross boundaries without an ACT bubble.
- Normalization per head: PSUM->SBUF copy (bf16), PE ones-outer-product to
  broadcast the denominator row, DVE fast reciprocal, DVE multiply.  Head 0
  writes O^T directly; head 1 goes through one partition-shifting
  SBUF->SBUF DMA.
"""

import numpy as np

P = 128
B, S, D = 2, 2048, 1024
H, DK = 16, 64
COLS = 256          # qkv columns per core (4 heads)
KC = D // P         # 8 contraction chunks for the projections
TT = 512            # token block (matmul free dim)
NJ = S // TT        # 4 token blocks
NT = S // P         # 16 token tiles
NKT = S // P        # 16 key tiles
VW = 65             # per-head AV lhsT width: 64 v-dims + ones column

_CACHE = {}


def _build():
    import concourse.bass as bass
    import concourse.tile as tile
    from concourse import bacc, mybir

    f32 = mybir.dt.float32
    bf16 = mybir.dt.bfloat16
    Exp = mybir.ActivationFunctionType.Exp

    nc = bacc.Bacc(
        "TRN2", target_bir_lowering=False, debug=False,
        enable_asserts=False, num_devices=8,
    )
    xt_d = nc.dram_tensor("xt", [D, S], bf16, kind="ExternalInput").ap()
    wq_d = nc.dram_tensor("wq", [D, COLS], bf16, kind="ExternalInput").ap()
    wk_d = nc.dram_tensor("wk", [D, COLS], bf16, kind="ExternalInput").ap()
    wv_d = nc.dram_tensor("wv", [D, COLS], bf16, kind="ExternalInput").ap()
    wo_d = nc.dram_tensor("wo", [COLS, D], bf16, kind="ExternalInput").ap()
    bq_d = nc.dram_tensor("bq", [P, 2], f32, kind="ExternalInput").ap()
    bk_d = nc.dram_tensor("bk", [P, 2], f32, kind="ExternalInput").ap()
    out_d = nc.dram_tensor("out_t", [D, S], bf16, kind="ExternalOutput").ap()

    with tile.TileContext(nc) as tc:
        with (
            tc.tile_pool(name="const", bufs=1) as const,
            tc.tile_pool(name="wpool", bufs=1) as wpool,
            tc.tile_pool(name="persist", bufs=1) as persist,
            tc.tile_pool(name="exps", bufs=4) as exps,
            tc.tile_pool(name="stage", bufs=4) as stage,
            tc.tile_pool(name="outst", bufs=4) as outst,
            tc.tile_pool(name="ps_sc", bufs=1, space="PSUM") as ps_sc,
            tc.tile_pool(name="ps_acc", bufs=1, space="PSUM") as ps_acc,
            tc.tile_pool(name="ps_u", bufs=2, space="PSUM") as ps_u,
        ):
            ones_b = const.tile([P, VW], bf16, tag="ones_b")
            nc.vector.memset(ones_b[:], 1.0)

            # warm the PE p-state (0.65 -> 2.4GHz ramps after ~3us of
            # continuous execution) with throwaway matmuls during the
            # otherwise-idle windows while the weight/x DMAs land; the
            # lead-in chains then run at full rate instead of ~390-630ns/mm
            dummy = const.tile([P, TT], bf16, tag="dummy")
            nc.vector.memset(dummy[:], 0.0)
            warm_box = {}

            def warm(n, fresh=False):
                if fresh or "t" not in warm_box:
                    warm_box["t"] = ps_u.tile([P, TT], f32, tag="u",
                                              name="warm")
                for _ in range(n):
                    nc.tensor.matmul(
                        warm_box["t"][0:VW, :], ones_b[:, :], dummy[:, :],
                        start=True, stop=True,
                    )

            warm(20)

            # ---- weights: already bf16 in DRAM (host-side cast); issued on
            # the scalar queue so they don't serialize behind the x DMAs on
            # the sync queue.  Biases come pre-shaped [128, 2] from the host
            # (a "(o p) -> p o" DRAM gather is 256 4-byte descriptors that
            # crawl through the DMA fabric) and load first — the first
            # K bias-add otherwise stalls the whole projection chain. ----
            # split wk: the first two chunks land fast so the very first
            # projection chain isn't gated on the full 0.5MB transfer
            wk_r0 = wpool.tile([P, 2, COLS], bf16, tag="w_k0")
            nc.scalar.dma_start(
                wk_r0[:], wk_d[0 : 2 * P, :].rearrange("(o p) f -> p o f", p=P)
            )
            # wk_r1 rides the sync queue ahead of the x tiles so it lands
            # in parallel with wk_r0 instead of serializing behind it
            wk_r1 = wpool.tile([P, KC - 2, COLS], bf16, tag="w_k1")
            nc.sync.dma_start(
                wk_r1[:], wk_d[2 * P :, :].rearrange("(o p) f -> p o f", p=P)
            )
            bq_sb = const.tile([P, 2], f32, tag="bq")
            nc.scalar.dma_start(bq_sb[:], bq_d)
            bk_sb = const.tile([P, 2], f32, tag="bk")
            nc.scalar.dma_start(bk_sb[:], bk_d)
            wv_r = wpool.tile([P, KC, COLS], bf16, tag="w_v")
            nc.scalar.dma_start(wv_r[:], wv_d.rearrange("(o p) f -> p o f", p=P))
            wq_r = wpool.tile([P, KC, COLS], bf16, tag="w_q")
            nc.scalar.dma_start(wq_r[:], wq_d.rearrange("(o p) f -> p o f", p=P))
            wo_r = wpool.tile([P, 2, D], bf16, tag="w_o")
            nc.scalar.dma_start(wo_r[:], wo_d.rearrange("(o p) f -> p o f", p=P))

            # persistent activations (all bf16)
            qT = persist.tile([P, 2, S], bf16, tag="qT")    # [qcol, tok]
            kT = persist.tile([P, 2, S], bf16, tag="kT")    # [kcol, tok]
            vt = persist.tile([P, NT, 4 * VW], bf16, tag="vt")  # [tok, h*(V|1)]
            oT = persist.tile([P, 2, S], bf16, tag="oT")    # [vdim, tok]
            xTs = [persist.tile([P, KC, TT], bf16, tag=f"xT{j}", name=f"xT{j}")
                   for j in range(NJ)]

            # ones column (index 64 of each head's VW slice)
            vt_heads = vt[:].rearrange("p t (h c) -> p t h c", c=VW)
            nc.vector.tensor_copy(
                vt_heads[:, :, :, 64],
                ones_b[:, :NT * 4].rearrange("p (t h) -> p t h", h=4),
            )

            xt_r = xt_d.rearrange("(o p) t -> p o t", p=P)
            for j in range(NJ):
                nc.sync.dma_start(xTs[j][:], xt_r[:, :, bass.ts(j, TT)])

            # ---- projection emitters ----
            def wk_sel(kc):
                return wk_r0[:, kc, :] if kc < 2 else wk_r1[:, kc - 2, :]

            def wq_sel(kc):
                return wq_r[:, kc, :]

            def qk_proj_ct(j, wsel, bsb, dstT, ct, acc, kc0, kc1):
                for kc in range(kc0, kc1):
                    nc.tensor.matmul(
                        acc[:], wsel(kc)[:, bass.ts(ct, P)], xTs[j][:, kc, :],
                        start=(kc == 0), stop=(kc == KC - 1),
                    )
                if kc1 == KC:
                    nc.vector.tensor_scalar_add(
                        dstT[:, ct, bass.ts(j, TT)], acc[:], bsb[:, ct : ct + 1]
                    )

            def v_proj(j, ts_):
                acc = ps_u.tile([P, COLS], f32, tag="u", name="v_acc")
                for kc in range(KC):
                    nc.tensor.matmul(
                        acc[:], xTs[j][:, kc, bass.ts(ts_, P)], wv_r[:, kc, :],
                        start=(kc == 0), stop=(kc == KC - 1),
                    )
                tt = 4 * j + ts_
                nc.vector.tensor_copy(
                    vt_heads[:, tt, :, 0:64],
                    acc[:].rearrange("p (h c) -> p h c", c=64),
                )

            # ---- phase A (lead-in): only what block (0,0) needs up front:
            # K(0) ct0, V(0), Q(0) ct0.  Everything else becomes PE filler
            # work inside the ACT-paced attention loop — PE is the global
            # bottleneck, so projection work must hide under the exp
            # stream instead of serializing before it. ----
            acc = ps_u.tile([P, TT], f32, tag="u", name="k_acc")
            qk_proj_ct(0, wk_sel, bk_sb, kT, 0, acc, 0, 2)
            warm(12)   # hold the p-state through the wk_r1 arrival window
            qk_proj_ct(0, wk_sel, bk_sb, kT, 0, acc, 2, KC)
            for ts_ in range(TT // P):
                v_proj(0, ts_)
            acc = ps_u.tile([P, TT], f32, tag="u", name="q_acc")
            qk_proj_ct(0, wq_sel, bq_sb, qT, 0, acc, 0, KC)

            # ---- filler queue: atomic groups of PE work (projection column
            # tiles, V units, out-projection units) drained a few steps per
            # kc slot inside the attention loop.  A multi-slot group is only
            # started when it fits in the current block's remaining slots,
            # so a ps_u accumulation never straddles the block boundary
            # where the norm's rbc tiles rotate through ps_u (that
            # interleaving could deadlock the in-order PE queue). ----
            fillers = []      # list of (key, [step closures])
            active = []       # remaining steps of the started group
            active_key = [None]
            done_keys = set()
            # produced in the lead-in:
            done_keys.update([("k", 0, 0), ("q", 0, 0)])
            done_keys.update([("v", 0, ts_) for ts_ in range(4)])

            def qkproj_group(j, ct, wsel, bsb, dstT, nm):
                box = {}
                def step(kc0, box=box):
                    if kc0 == 0:
                        box["acc"] = ps_u.tile([P, TT], f32, tag="u", name=nm)
                    qk_proj_ct(j, wsel, bsb, dstT, ct, box["acc"],
                               kc0, kc0 + 1)
                return [lambda kc0=kc0: step(kc0) for kc0 in range(KC)]

            def vproj_group(j, ts_):
                box = {}
                def step(kc0, box=box):
                    if kc0 == 0:
                        box["acc"] = ps_u.tile([P, COLS], f32, tag="u",
                                               name="v_acc")
                    acc = box["acc"]
                    for kc in range(kc0, kc0 + 2):
                        nc.tensor.matmul(
                            acc[:], xTs[j][:, kc, bass.ts(ts_, P)],
                            wv_r[:, kc, :],
                            start=(kc == 0), stop=(kc == KC - 1),
                        )
                    if kc0 == KC - 2:
                        tt = 4 * j + ts_
                        nc.vector.tensor_copy(
                            vt_heads[:, tt, :, 0:64],
                            acc[:].rearrange("p (h c) -> p h c", c=64),
                        )
                return [lambda kc0=kc0: step(kc0) for kc0 in range(0, KC, 2)]

            tail_acc_box = {}

            def outproj_group(j, oc):
                box = {}

                def s1():
                    if j == NJ - 1 and oc % 4 >= 2:
                        # tail units: the attention o_ps banks are free
                        # after the final norm copies — borrow them so the
                        # PSUM rotation is 4-deep instead of 2-deep
                        if oc % 4 == 2:
                            tail_acc_box["t"] = ps_acc.tile(
                                [P, 2, TT], f32, tag="acc", name="tail_acc"
                            )
                        box["acc"] = tail_acc_box["t"][:, oc % 2, :]
                    else:
                        box["acc"] = ps_u.tile([P, TT], f32, tag="u",
                                               name="wo_acc")[:]
                    nc.tensor.matmul(
                        box["acc"], wo_r[:, 0, bass.ts(oc, P)],
                        oT[:, 0, bass.ts(j, TT)], start=True, stop=False,
                    )

                def s2():
                    acc = box["acc"]
                    nc.tensor.matmul(
                        acc, wo_r[:, 1, bass.ts(oc, P)],
                        oT[:, 1, bass.ts(j, TT)], start=False, stop=True,
                    )
                    st = outst.tile([P, TT], bf16, tag="outst", name="outst")
                    if j == NJ - 1 and oc % 2 == 0:
                        # the last j's units drain after the final exp:
                        # alternate the then-idle ACT engine with DVE so the
                        # tail's PSUM->SBUF copies run on two engines
                        nc.scalar.copy(st[:], acc)
                    else:
                        nc.vector.tensor_copy(st[:], acc)
                    nc.sync.dma_start(
                        out_d[bass.ts(oc, P), bass.ts(j, TT)], st[:]
                    )

                return [s1, s2]

            def _finish_active():
                while active:
                    active.pop(0)()
                if active_key[0] is not None:
                    done_keys.add(active_key[0])
                    active_key[0] = None

            def drain_filler(slots_left, n=1):
                for _ in range(n):
                    if not active:
                        if active_key[0] is not None:
                            done_keys.add(active_key[0])
                            active_key[0] = None
                        for gi, (key, grp) in enumerate(fillers):
                            if len(grp) <= slots_left:
                                key, grp = fillers.pop(gi)
                                active.extend(grp)
                                active_key[0] = key
                                break
                        else:
                            return
                    active.pop(0)()
                if not active and active_key[0] is not None:
                    done_keys.add(active_key[0])
                    active_key[0] = None

            def ensure(key):
                # force-emit producer groups (in queue order) until `key`
                # has been fully emitted.  Called before the consumer is
                # emitted so the dependency is recorded.
                if key in done_keys:
                    return
                if active_key[0] == key:
                    _finish_active()
                    return
                while key not in done_keys:
                    _finish_active()
                    if not fillers:
                        raise RuntimeError(f"missing producer {key}")
                    k, grp = fillers.pop(0)
                    active.extend(grp)
                    active_key[0] = k
                _finish_active()

            # production order: per j, the K/Q ct0 and V needed by the p=0
            # blocks; then all ct1 work needed by the p=1 blocks.
            for j in range(1, NJ):
                fillers.append((("k", j, 0),
                                qkproj_group(j, 0, wk_sel, bk_sb, kT, "k_acc")))
                fillers.append((("q", j, 0),
                                qkproj_group(j, 0, wq_sel, bq_sb, qT, "q_acc")))
                for ts_ in range(TT // P):
                    fillers.append((("v", j, ts_), vproj_group(j, ts_)))
            for j in range(NJ):
                fillers.append((("k", j, 1),
                                qkproj_group(j, 1, wk_sel, bk_sb, kT, "k_acc")))
            for j in range(NJ):
                fillers.append((("q", j, 1),
                                qkproj_group(j, 1, wq_sel, bq_sb, qT, "q_acc")))

            # ---- normalization, split in two parts: the PSUM->SBUF copies
            # (the only o_ps reads) are emitted right after the last AV so
            # the next block's o_ps alloc records them; the arithmetic runs
            # after the next block's first scores so PE/ACT keep flowing ----
            def norm_copies(o_ps):
                osbs = []
                for i in range(2):
                    osb = stage.tile([VW, TT], bf16, tag="osb", name="osb")
                    nc.vector.tensor_copy(osb[:], o_ps[0:VW, i, :])
                    osbs.append(osb)
                return osbs

            def norm_arith(j, p, osbs):
                for i in range(2):
                    osb = osbs[i]
                    rbc = ps_u.tile([64, TT], f32, tag="u", name="rbc")
                    nc.tensor.matmul(
                        rbc[:], ones_b[64:65, 0:64], osb[64:65, :],
                        start=True, stop=True,
                    )
                    rbs = stage.tile([64, TT], f32, tag="rbs", name="rbs")
                    nc.vector.reciprocal_approx_fast(rbs[:], rbc[:])
                    if i == 0:
                        nc.vector.tensor_tensor(
                            oT[0:64, p, bass.ts(j, TT)], osb[0:64, :], rbs[:],
                            mybir.AluOpType.mult,
                        )
                    else:
                        onrm = stage.tile([64, TT], bf16, tag="onrm",
                                          name="onrm")
                        nc.vector.tensor_tensor(
                            onrm[:], osb[0:64, :], rbs[:], mybir.AluOpType.mult
                        )
                        nc.sync.dma_start(
                            oT[64:128, p, bass.ts(j, TT)], onrm[:]
                        )

            # scores PSUM: two parity tiles of 2 banks each.  Separate tiles
            # (not one [P,4,TT] tensor) so the tile-granular WAR dependency
            # lets sc(kc+2) overlap exp(kc+1): one tile would serialize every
            # score matmul behind the latest exp read, collapsing the
            # pipeline to 1-deep (measured 1.66us/kc vs ACT's 1.11us).
            big_scs = [ps_sc.tile([P, 2, TT], f32, tag=f"sc{par}",
                                  name=f"sc{par}") for par in range(2)]

            # ---- attention: ACT-paced kc pipeline with PE fillers.  The
            # next block's first two score pairs are emitted during the
            # current block's last two kc slots so the exp stream crosses
            # block boundaries without a bubble. ----
            blocks = [(j, p) for p in range(2) for j in range(NJ)]

            def sc_emit_b(t, kc):
                j, p = blocks[t]
                ensure(("k", kc // 4, p))
                ensure(("q", j, p))
                sc = big_scs[kc % 2]
                for i in range(2):
                    lo, hi = 64 * i, 64 * i + 64
                    nc.tensor.matmul(
                        sc[:, i, :],
                        kT[lo:hi, p, bass.ts(kc, P)],
                        qT[lo:hi, p, bass.ts(j, TT)],
                        start=True, stop=True,
                    )

            pending_norm = None
            for t, (j, p) in enumerate(blocks):
                o_ps = ps_acc.tile([P, 2, TT], f32, tag="acc", name="o_ps")

                def av_emit(kc, ex, p=p, o_ps=o_ps):
                    ensure(("v", kc // 4, kc % 4))
                    for i in range(2):
                        h = 2 * p + i
                        nc.tensor.matmul(
                            o_ps[0:VW, i, :],
                            vt[:, kc, bass.ds(VW * h, VW)],
                            ex[:, i, :],
                            start=(kc == 0), stop=(kc == NKT - 1),
                        )

                if t == 0:
                    sc_emit_b(0, 0)
                    sc_emit_b(0, 1)
                if pending_norm is not None:
                    pending_norm()
                    pending_norm = None
                # block 0 must mass-produce K/V for its own kc stream; later
                # p=0 blocks only need their own Q/K ct0 (force-ensured), so
                # defer the rest into the ACT-paced slack of the p=1 blocks
                ndrain = 3 if t == 0 else 1
                prev = None
                for kc in range(NKT):
                    ex = exps.tile([P, 2, TT], bf16, tag="exp", name="ex")
                    nc.scalar.activation(
                        ex[:], big_scs[kc % 2][:], Exp, scale=0.125,
                    )
                    drain_filler(NKT - kc, ndrain)
                    if prev is not None:
                        av_emit(kc - 1, prev)
                    if kc + 2 < NKT:
                        sc_emit_b(t, kc + 2)
                    elif t + 1 < len(blocks):
                        sc_emit_b(t + 1, kc - (NKT - 2))
                    prev = ex
                av_emit(NKT - 1, prev)
                osbs = norm_copies(o_ps)
                pending_norm = (
                    lambda j=j, p=p, osbs=osbs: norm_arith(j, p, osbs)
                )
                if p == 1:
                    for oc in range(D // P):
                        fillers.append((("o", j, oc), outproj_group(j, oc)))
            pending_norm()
            # keep the PE p-state up through the final norm's DVE chain so
            # the tail out-projection matmuls run warm (measured 634ns/mm
            # after a ~2.5us PE gap here, vs 241ns warm)
            warm(8, fresh=True)
            while fillers or active:
                drain_filler(NKT)

    nc.compile()
    return nc


def make_in_maps(x, Wq, bq, Wk, bk, Wv, Wo):
    import ml_dtypes

    bf = ml_dtypes.bfloat16
    xt = [np.ascontiguousarray(x[b].T.astype(bf)) for b in range(B)]

    in_maps = []
    for c in range(8):
        b, g = divmod(c, 4)
        cs = slice(COLS * g, COLS * (g + 1))
        in_maps.append({
            "xt": xt[b],
            "wq": np.ascontiguousarray(Wq[:, cs].astype(bf)),
            "wk": np.ascontiguousarray(Wk[:, cs].astype(bf)),
            "wv": np.ascontiguousarray(Wv[:, cs].astype(bf)),
            "wo": np.ascontiguousarray(Wo[cs, :].astype(bf)),
            "bq": np.ascontiguousarray(bq[cs].reshape(2, P).T),
            "bk": np.ascontiguousarray(bk[cs].reshape(2, P).T),
        })
    return in_maps


def kernel(x, Wq, bq, Wk, bk, Wv, bv, Wo, bo):
    from concourse import bass_utils

    x = np.asarray(x, dtype=np.float32)
    Wq = np.asarray(Wq, dtype=np.float32)
    Wk = np.asarray(Wk, dtype=np.float32)
    Wv = np.asarray(Wv, dtype=np.float32)
    Wo = np.asarray(Wo, dtype=np.float32)
    bq = np.asarray(bq, dtype=np.float32)
    bk = np.asarray(bk, dtype=np.float32)
    bv = np.asarray(bv, dtype=np.float32)
    bo = np.asarray(bo, dtype=np.float32)

    if "nc" not in _CACHE:
        _CACHE["nc"] = _build()
    nc = _CACHE["nc"]

    in_maps = make_in_maps(x, Wq, bq, Wk, bk, Wv, Wo)
    res = bass_utils.run_bass_kernel_spmd(nc, in_maps, core_ids=list(range(8)))

    out = np.zeros((B, S, D), dtype=np.float32)
    for c in range(8):
        out[c // 4] += res.results[c]["out_t"].T.astype(np.float32)
    out += bo + bv @ Wo
    return out


# revision 66
# speedup vs baseline: 1.0035x; 1.0035x over previous
"""Multi-head attention (B=2, S=2048, D=1024, H=16, dk=64) on 8 Trainium2
NeuronCores via Bass/Tile.

Sharding: core c handles batch b = c//4 and head-group g = c%4 (4 heads,
256 qkv columns).  Each core computes its QKV projection slices, 4 heads of
attention, and a partial output projection against its 256-row slice of Wo.
The host sums the 4 partial outputs per batch (row-sharded Wo => partial
sums) and folds in the biases bo and bv@Wo (softmax rows sum to 1, so the
V-bias contributes exactly bv@Wo per token).

v3 design notes (vs v2's f32r):
- Everything bf16: fp32 feeds the PE at half the bf16 streaming rate
  (measured 1.2 rows/ns vs 2.4), so bf16 operands double matmul throughput.
  PSUM accumulation stays f32.  Measured numeric impact ~2.3e-3 rel_l2.
- Weights are cast to bf16 host-side -> straight DMA, no on-chip casts.
- x is DMA-transposed once (bf16, no hi/lo split).
- The exp on the ACT engine (~1.1us per [128,2,512] chunk) paces the
  attention phase; the PE (~182us busy) is the global bottleneck.  Only
  K(0)/V(0)/Q(0) of the p=0 column tile run as a lead-in; every other
  projection column-tile, V unit, and output-projection unit is emitted as
  "filler" PE work inside the attention kc loop (demand-driven via
  ensure(), which force-emits producer groups before their consumers so
  dependencies are recorded), hiding under the exp stream instead of
  serializing before/after it.  Scores PSUM is two parity tiles so the
  tile-granular WAR dep doesn't collapse the sc->exp pipeline to 1-deep,
  and each block's first two score pairs are emitted during the previous
  block's last two slots to cross boundaries without an ACT bubble.
- Normalization per head: PSUM->SBUF copy (bf16), PE ones-outer-product to
  broadcast the denominator row, DVE fast reciprocal, DVE multiply.  Head 0
  writes O^T directly; head 1 goes through one partition-shifting
  SBUF->SBUF DMA.
"""

import numpy as np

P = 128
B, S, D = 2, 2048, 1024
H, DK = 16, 64
COLS = 256          # qkv columns per core (4 heads)
KC = D // P         # 8 contraction chunks for the projections
TT = 512            # token block (matmul free dim)
NJ = S // TT        # 4 token blocks
NT = S // P         # 16 token tiles
NKT = S // P        # 16 key tiles
VW = 65             # per-head AV lhsT width: 64 v-dims + ones column

_CACHE = {}


def _build():
    import concourse.bass as bass
    import concourse.tile as tile
    from concourse import bacc, mybir

    f32 = mybir.dt.float32
    bf16 = mybir.dt.bfloat16
    Exp = mybir.ActivationFunctionType.Exp

    nc = bacc.Bacc(
        "TRN2", target_bir_lowering=False, debug=False,
        enable_asserts=False, num_devices=8,
    )
    xt_d = nc.dram_tensor("xt", [D, S], bf16, kind="ExternalInput").ap()
    wq_d = nc.dram_tensor("wq", [D, COLS], bf16, kind="ExternalInput").ap()
    wk_d = nc.dram_tensor("wk", [D, COLS], bf16, kind="ExternalInput").ap()
    wv_d = nc.dram_tensor("wv", [D, COLS], bf16, kind="ExternalInput").ap()
    wo_d = nc.dram_tensor("wo", [COLS, D], bf16, kind="ExternalInput").ap()
    bq_d = nc.dram_tensor("bq", [P, 2], f32, kind="ExternalInput").ap()
    bk_d = nc.dram_tensor("bk", [P, 2], f32, kind="ExternalInput").ap()
    out_d = nc.dram_tensor("out_t", [D, S], bf16, kind="ExternalOutput").ap()

    with tile.TileContext(nc) as tc:
        with (
            tc.tile_pool(name="const", bufs=1) as const,
            tc.tile_pool(name="wpool", bufs=1) as wpool,
            tc.tile_pool(name="persist", bufs=1) as persist,
            tc.tile_pool(name="exps", bufs=3) as exps,
            tc.tile_pool(name="stage", bufs=4) as stage,
            tc.tile_pool(name="outst", bufs=4) as outst,
            tc.tile_pool(name="ps_sc", bufs=1, space="PSUM") as ps_sc,
            tc.tile_pool(name="ps_acc", bufs=1, space="PSUM") as ps_acc,
            tc.tile_pool(name="ps_u", bufs=2, space="PSUM") as ps_u,
        ):
            ones_b = const.tile([P, VW], bf16, tag="ones_b")
            nc.vector.memset(ones_b[:], 1.0)

            # warm the PE p-state (0.65 -> 2.4GHz ramps after ~3us of
            # continuous execution) with throwaway matmuls during the
            # otherwise-idle windows while the weight/x DMAs land; the
            # lead-in chains then run at full rate instead of ~390-630ns/mm
            dummy = const.tile([P, TT], bf16, tag="dummy")
            nc.vector.memset(dummy[:], 0.0)
            warm_box = {}

            def warm(n, fresh=False):
                if fresh or "t" not in warm_box:
                    warm_box["t"] = ps_u.tile([P, TT], f32, tag="u",
                                              name="warm")
                for _ in range(n):
                    nc.tensor.matmul(
                        warm_box["t"][0:VW, :], ones_b[:, :], dummy[:, :],
                        start=True, stop=True,
                    )

            warm(20)

            # ---- weights: already bf16 in DRAM (host-side cast); issued on
            # the scalar queue so they don't serialize behind the x DMAs on
            # the sync queue.  Biases come pre-shaped [128, 2] from the host
            # (a "(o p) -> p o" DRAM gather is 256 4-byte descriptors that
            # crawl through the DMA fabric) and load first — the first
            # K bias-add otherwise stalls the whole projection chain. ----
            # split wk: the first two chunks land fast so the very first
            # projection chain isn't gated on the full 0.5MB transfer
            wk_r0 = wpool.tile([P, 2, COLS], bf16, tag="w_k0")
            nc.scalar.dma_start(
                wk_r0[:], wk_d[0 : 2 * P, :].rearrange("(o p) f -> p o f", p=P)
            )
            # wk_r1 rides the sync queue ahead of the x tiles so it lands
            # in parallel with wk_r0 instead of serializing behind it
            wk_r1 = wpool.tile([P, KC - 2, COLS], bf16, tag="w_k1")
            nc.sync.dma_start(
                wk_r1[:], wk_d[2 * P :, :].rearrange("(o p) f -> p o f", p=P)
            )
            bq_sb = const.tile([P, 2], f32, tag="bq")
            nc.scalar.dma_start(bq_sb[:], bq_d)
            bk_sb = const.tile([P, 2], f32, tag="bk")
            nc.scalar.dma_start(bk_sb[:], bk_d)
            wv_r = wpool.tile([P, KC, COLS], bf16, tag="w_v")
            nc.scalar.dma_start(wv_r[:], wv_d.rearrange("(o p) f -> p o f", p=P))
            wq_r = wpool.tile([P, KC, COLS], bf16, tag="w_q")
            nc.scalar.dma_start(wq_r[:], wq_d.rearrange("(o p) f -> p o f", p=P))
            wo_r = wpool.tile([P, 2, D], bf16, tag="w_o")
            nc.scalar.dma_start(wo_r[:], wo_d.rearrange("(o p) f -> p o f", p=P))

            # persistent activations (all bf16)
            qT = persist.tile([P, 2, S], bf16, tag="qT")    # [qcol, tok]
            kT = persist.tile([P, 2, S], bf16, tag="kT")    # [kcol, tok]
            vt = persist.tile([P, NT, 4 * VW], bf16, tag="vt")  # [tok, h*(V|1)]
            oT = persist.tile([P, 2, S], bf16, tag="oT")    # [vdim, tok]
            xTs = [persist.tile([P, KC, TT], bf16, tag=f"xT{j}", name=f"xT{j}")
                   for j in range(NJ)]

            # ones column (index 64 of each head's VW slice)
            vt_heads = vt[:].rearrange("p t (h c) -> p t h c", c=VW)
            nc.vector.tensor_copy(
                vt_heads[:, :, :, 64],
                ones_b[:, :NT * 4].rearrange("p (t h) -> p t h", h=4),
            )

            xt_r = xt_d.rearrange("(o p) t -> p o t", p=P)
            for j in range(NJ):
                nc.sync.dma_start(xTs[j][:], xt_r[:, :, bass.ts(j, TT)])

            # ---- projection emitters ----
            def wk_sel(kc):
                return wk_r0[:, kc, :] if kc < 2 else wk_r1[:, kc - 2, :]

            def wq_sel(kc):
                return wq_r[:, kc, :]

            def qk_proj_ct(j, wsel, bsb, dstT, ct, acc, kc0, kc1):
                for kc in range(kc0, kc1):
                    nc.tensor.matmul(
                        acc[:], wsel(kc)[:, bass.ts(ct, P)], xTs[j][:, kc, :],
                        start=(kc == 0), stop=(kc == KC - 1),
                    )
                if kc1 == KC:
                    nc.vector.tensor_scalar_add(
                        dstT[:, ct, bass.ts(j, TT)], acc[:], bsb[:, ct : ct + 1]
                    )

            def v_proj(j, ts_):
                acc = ps_u.tile([P, COLS], f32, tag="u", name="v_acc")
                for kc in range(KC):
                    nc.tensor.matmul(
                        acc[:], xTs[j][:, kc, bass.ts(ts_, P)], wv_r[:, kc, :],
                        start=(kc == 0), stop=(kc == KC - 1),
                    )
                tt = 4 * j + ts_
                nc.vector.tensor_copy(
                    vt_heads[:, tt, :, 0:64],
                    acc[:].rearrange("p (h c) -> p h c", c=64),
                )

            # ---- phase A (lead-in): only what block (0,0) needs up front:
            # K(0) ct0, V(0), Q(0) ct0.  Everything else becomes PE filler
            # work inside the ACT-paced attention loop — PE is the global
            # bottleneck, so projection work must hide under the exp
            # stream instead of serializing before it. ----
            acc = ps_u.tile([P, TT], f32, tag="u", name="k_acc")
            qk_proj_ct(0, wk_sel, bk_sb, kT, 0, acc, 0, 2)
            warm(12)   # hold the p-state through the wk_r1 arrival window
            qk_proj_ct(0, wk_sel, bk_sb, kT, 0, acc, 2, KC)
            for ts_ in range(TT // P):
                v_proj(0, ts_)
            acc = ps_u.tile([P, TT], f32, tag="u", name="q_acc")
            qk_proj_ct(0, wq_sel, bq_sb, qT, 0, acc, 0, KC)

            # ---- filler queue: atomic groups of PE work (projection column
            # tiles, V units, out-projection units) drained a few steps per
            # kc slot inside the attention loop.  A multi-slot group is only
            # started when it fits in the current block's remaining slots,
            # so a ps_u accumulation never straddles the block boundary
            # where the norm's rbc tiles rotate through ps_u (that
            # interleaving could deadlock the in-order PE queue). ----
            fillers = []      # list of (key, [step closures])
            active = []       # remaining steps of the started group
            active_key = [None]
            done_keys = set()
            # produced in the lead-in:
            done_keys.update([("k", 0, 0), ("q", 0, 0)])
            done_keys.update([("v", 0, ts_) for ts_ in range(4)])

            def qkproj_group(j, ct, wsel, bsb, dstT, nm):
                box = {}
                def step(kc0, box=box):
                    if kc0 == 0:
                        box["acc"] = ps_u.tile([P, TT], f32, tag="u", name=nm)
                    qk_proj_ct(j, wsel, bsb, dstT, ct, box["acc"],
                               kc0, kc0 + 1)
                return [lambda kc0=kc0: step(kc0) for kc0 in range(KC)]

            def vproj_group(j, ts_):
                box = {}
                def step(kc0, box=box):
                    if kc0 == 0:
                        box["acc"] = ps_u.tile([P, COLS], f32, tag="u",
                                               name="v_acc")
                    acc = box["acc"]
                    for kc in range(kc0, kc0 + 2):
                        nc.tensor.matmul(
                            acc[:], xTs[j][:, kc, bass.ts(ts_, P)],
                            wv_r[:, kc, :],
                            start=(kc == 0), stop=(kc == KC - 1),
                        )
                    if kc0 == KC - 2:
                        tt = 4 * j + ts_
                        nc.vector.tensor_copy(
                            vt_heads[:, tt, :, 0:64],
                            acc[:].rearrange("p (h c) -> p h c", c=64),
                        )
                return [lambda kc0=kc0: step(kc0) for kc0 in range(0, KC, 2)]

            tail_acc_box = {}

            def outproj_group(j, oc):
                box = {}

                def s1():
                    if j == NJ - 1 and oc % 4 >= 2:
                        # tail units: the attention o_ps banks are free
                        # after the final norm copies — borrow them so the
                        # PSUM rotation is 4-deep instead of 2-deep
                        if oc % 4 == 2:
                            tail_acc_box["t"] = ps_acc.tile(
                                [P, 2, TT], f32, tag="acc", name="tail_acc"
                            )
                        box["acc"] = tail_acc_box["t"][:, oc % 2, :]
                    else:
                        box["acc"] = ps_u.tile([P, TT], f32, tag="u",
                                               name="wo_acc")[:]
                    nc.tensor.matmul(
                        box["acc"], wo_r[:, 0, bass.ts(oc, P)],
                        oT[:, 0, bass.ts(j, TT)], start=True, stop=False,
                    )

                def s2():
                    acc = box["acc"]
                    nc.tensor.matmul(
                        acc, wo_r[:, 1, bass.ts(oc, P)],
                        oT[:, 1, bass.ts(j, TT)], start=False, stop=True,
                    )
                    st = outst.tile([P, TT], bf16, tag="outst", name="outst")
                    if j == NJ - 1 and oc % 2 == 0:
                        # the last j's units drain after the final exp:
                        # alternate the then-idle ACT engine with DVE so the
                        # tail's PSUM->SBUF copies run on two engines
                        nc.scalar.copy(st[:], acc)
                    else:
                        nc.vector.tensor_copy(st[:], acc)
                    nc.sync.dma_start(
                        out_d[bass.ts(oc, P), bass.ts(j, TT)], st[:]
                    )

                return [s1, s2]

            def _finish_active():
                while active:
                    active.pop(0)()
                if active_key[0] is not None:
                    done_keys.add(active_key[0])
                    active_key[0] = None

            def drain_filler(slots_left, n=1):
                for _ in range(n):
                    if not active:
                        if active_key[0] is not None:
                            done_keys.add(active_key[0])
                            active_key[0] = None
                        for gi, (key, grp) in enumerate(fillers):
                            if len(grp) <= slots_left:
                                key, grp = fillers.pop(gi)
                                active.extend(grp)
                                active_key[0] = key
                                break
                        else:
                            return
                    active.pop(0)()
                if not active and active_key[0] is not None:
                    done_keys.add(active_key[0])
                    active_key[0] = None

            def ensure(key):
                # force-emit producer groups (in queue order) until `key`
                # has been fully emitted.  Called before the consumer is
                # emitted so the dependency is recorded.
                if key in done_keys:
                    return
                if active_key[0] == key:
                    _finish_active()
                    return
                while key not in done_keys:
                    _finish_active()
                    if not fillers:
                        raise RuntimeError(f"missing producer {key}")
                    k, grp = fillers.pop(0)
                    active.extend(grp)
                    active_key[0] = k
                _finish_active()

            # production order: per j, the K/Q ct0 and V needed by the p=0
            # blocks; then all ct1 work needed by the p=1 blocks.
            for j in range(1, NJ):
                fillers.append((("k", j, 0),
                                qkproj_group(j, 0, wk_sel, bk_sb, kT, "k_acc")))
                fillers.append((("q", j, 0),
                                qkproj_group(j, 0, wq_sel, bq_sb, qT, "q_acc")))
                for ts_ in range(TT // P):
                    fillers.append((("v", j, ts_), vproj_group(j, ts_)))
            for j in range(NJ):
                fillers.append((("k", j, 1),
                                qkproj_group(j, 1, wk_sel, bk_sb, kT, "k_acc")))
            for j in range(NJ):
                fillers.append((("q", j, 1),
                                qkproj_group(j, 1, wq_sel, bq_sb, qT, "q_acc")))

            # ---- normalization, split in two parts: the PSUM->SBUF copies
            # (the only o_ps reads) are emitted right after the last AV so
            # the next block's o_ps alloc records them; the arithmetic runs
            # after the next block's first scores so PE/ACT keep flowing ----
            def norm_copies(o_ps):
                osbs = []
                for i in range(2):
                    osb = stage.tile([VW, TT], bf16, tag="osb", name="osb")
                    nc.vector.tensor_copy(osb[:], o_ps[0:VW, i, :])
                    osbs.append(osb)
                return osbs

            def norm_arith(j, p, osbs):
                for i in range(2):
                    osb = osbs[i]
                    rbc = ps_u.tile([64, TT], f32, tag="u", name="rbc")
                    nc.tensor.matmul(
                        rbc[:], ones_b[64:65, 0:64], osb[64:65, :],
                        start=True, stop=True,
                    )
                    rbs = stage.tile([64, TT], f32, tag="rbs", name="rbs")
                    nc.vector.reciprocal_approx_fast(rbs[:], rbc[:])
                    if i == 0:
                        nc.vector.tensor_tensor(
                            oT[0:64, p, bass.ts(j, TT)], osb[0:64, :], rbs[:],
                            mybir.AluOpType.mult,
                        )
                    else:
                        onrm = stage.tile([64, TT], bf16, tag="onrm",
                                          name="onrm")
                        nc.vector.tensor_tensor(
                            onrm[:], osb[0:64, :], rbs[:], mybir.AluOpType.mult
                        )
                        nc.sync.dma_start(
                            oT[64:128, p, bass.ts(j, TT)], onrm[:]
                        )

            # scores PSUM: two parity tiles of 2 banks each.  Separate tiles
            # (not one [P,4,TT] tensor) so the tile-granular WAR dependency
            # lets sc(kc+2) overlap exp(kc+1): one tile would serialize every
            # score matmul behind the latest exp read, collapsing the
            # pipeline to 1-deep (measured 1.66us/kc vs ACT's 1.11us).
            big_scs = [ps_sc.tile([P, 2, TT], f32, tag=f"sc{par}",
                                  name=f"sc{par}") for par in range(2)]

            # ---- attention: ACT-paced kc pipeline with PE fillers.  The
            # next block's first two score pairs are emitted during the
            # current block's last two kc slots so the exp stream crosses
            # block boundaries without a bubble. ----
            blocks = [(j, p) for p in range(2) for j in range(NJ)]

            def sc_emit_b(t, kc):
                j, p = blocks[t]
                ensure(("k", kc // 4, p))
                ensure(("q", j, p))
                sc = big_scs[kc % 2]
                for i in range(2):
                    lo, hi = 64 * i, 64 * i + 64
                    nc.tensor.matmul(
                        sc[:, i, :],
                        kT[lo:hi, p, bass.ts(kc, P)],
                        qT[lo:hi, p, bass.ts(j, TT)],
                        start=True, stop=True,
                    )

            pending_norm = None
            for t, (j, p) in enumerate(blocks):
                o_ps = ps_acc.tile([P, 2, TT], f32, tag="acc", name="o_ps")

                def av_emit(kc, ex, p=p, o_ps=o_ps):
                    ensure(("v", kc // 4, kc % 4))
                    for i in range(2):
                        h = 2 * p + i
                        nc.tensor.matmul(
                            o_ps[0:VW, i, :],
                            vt[:, kc, bass.ds(VW * h, VW)],
                            ex[:, i, :],
                            start=(kc == 0), stop=(kc == NKT - 1),
                        )

                if t == 0:
                    sc_emit_b(0, 0)
                    sc_emit_b(0, 1)
                if pending_norm is not None:
                    pending_norm()
                    pending_norm = None
                # block 0 must mass-produce K/V for its own kc stream; later
                # p=0 blocks only need their own Q/K ct0 (force-ensured), so
                # defer the rest into the ACT-paced slack of the p=1 blocks
                ndrain = 3 if t == 0 else 1
                prev = None
                for kc in range(NKT):
                    ex = exps.tile([P, 2, TT], bf16, tag="exp", name="ex")
                    nc.scalar.activation(
                        ex[:], big_scs[kc % 2][:], Exp, scale=0.125,
                    )
                    drain_filler(NKT - kc, ndrain)
                    if prev is not None:
                        av_emit(kc - 1, prev)
                    if kc + 2 < NKT:
                        sc_emit_b(t, kc + 2)
                    elif t + 1 < len(blocks):
                        sc_emit_b(t + 1, kc - (NKT - 2))
                    prev = ex
                av_emit(NKT - 1, prev)
                osbs = norm_copies(o_ps)
                pending_norm = (
                    lambda j=j, p=p, osbs=osbs: norm_arith(j, p, osbs)
                )
                if p == 1:
                    for oc in range(D // P):
                        fillers.append((("o", j, oc), outproj_group(j, oc)))
            pending_norm()
            # keep the PE p-state up through the final norm's DVE chain so
            # the tail out-projection matmuls run warm (measured 634ns/mm
            # after a ~2.5us PE gap here, vs 241ns warm)
            warm(8, fresh=True)
            while fillers or active:
                drain_filler(NKT)

    nc.compile()
    return nc


def make_in_maps(x, Wq, bq, Wk, bk, Wv, Wo):
    import ml_dtypes

    bf = ml_dtypes.bfloat16
    xt = [np.ascontiguousarray(x[b].T.astype(bf)) for b in range(B)]

    in_maps = []
    for c in range(8):
        b, g = divmod(c, 4)
        cs = slice(COLS * g, COLS * (g + 1))
        in_maps.append({
            "xt": xt[b],
            "wq": np.ascontiguousarray(Wq[:, cs].astype(bf)),
            "wk": np.ascontiguousarray(Wk[:, cs].astype(bf)),
            "wv": np.ascontiguousarray(Wv[:, cs].astype(bf)),
            "wo": np.ascontiguousarray(Wo[cs, :].astype(bf)),
            "bq": np.ascontiguousarray(bq[cs].reshape(2, P).T),
            "bk": np.ascontiguousarray(bk[cs].reshape(2, P).T),
        })
    return in_maps


def kernel(x, Wq, bq, Wk, bk, Wv, bv, Wo, bo):
    from concourse import bass_utils

    x = np.asarray(x, dtype=np.float32)
    Wq = np.asarray(Wq, dtype=np.float32)
    Wk = np.asarray(Wk, dtype=np.float32)
    Wv = np.asarray(Wv, dtype=np.float32)
    Wo = np.asarray(Wo, dtype=np.float32)
    bq = np.asarray(bq, dtype=np.float32)
    bk = np.asarray(bk, dtype=np.float32)
    bv = np.asarray(bv, dtype=np.float32)
    bo = np.asarray(bo, dtype=np.float32)

    if "nc" not in _CACHE:
        _CACHE["nc"] = _build()
    nc = _CACHE["nc"]

    in_maps = make_in_maps(x, Wq, bq, Wk, bk, Wv, Wo)
    res = bass_utils.run_bass_kernel_spmd(nc, in_maps, core_ids=list(range(8)))

    out = np.zeros((B, S, D), dtype=np.float32)
    for c in range(8):
        out[c // 4] += res.results[c]["out_t"].T.astype(np.float32)
    out += bo + bv @ Wo
    return out


# revision 67
# speedup vs baseline: 1.0118x; 1.0082x over previous
"""Multi-head attention (B=2, S=2048, D=1024, H=16, dk=64) on 8 Trainium2
NeuronCores via Bass/Tile.

Sharding: core c handles batch b = c//4 and head-group g = c%4 (4 heads,
256 qkv columns).  Each core computes its QKV projection slices, 4 heads of
attention, and a partial output projection against its 256-row slice of Wo.
The host sums the 4 partial outputs per batch (row-sharded Wo => partial
sums) and folds in the biases bo and bv@Wo (softmax rows sum to 1, so the
V-bias contributes exactly bv@Wo per token).

v3 design notes (vs v2's f32r):
- Everything bf16: fp32 feeds the PE at half the bf16 streaming rate
  (measured 1.2 rows/ns vs 2.4), so bf16 operands double matmul throughput.
  PSUM accumulation stays f32.  Measured numeric impact ~2.3e-3 rel_l2.
- Weights are cast to bf16 host-side -> straight DMA, no on-chip casts.
- x is DMA-transposed once (bf16, no hi/lo split).
- The exp on the ACT engine (~1.1us per [128,2,512] chunk) paces the
  attention phase; the PE (~182us busy) is the global bottleneck.  Only
  K(0)/V(0)/Q(0) of the p=0 column tile run as a lead-in; every other
  projection column-tile, V unit, and output-projection unit is emitted as
  "filler" PE work inside the attention kc loop (demand-driven via
  ensure(), which force-emits producer groups before their consumers so
  dependencies are recorded), hiding under the exp stream instead of
  serializing before/after it.  Scores PSUM is two parity tiles so the
  tile-granular WAR dep doesn't collapse the sc->exp pipeline to 1-deep,
  and each block's first two score pairs are emitted during the previous
  block's last two slots to cross boundaries without an ACT bubble.
- Normalization per head: PSUM->SBUF copy (bf16), PE ones-outer-product to
  broadcast the denominator row, DVE fast reciprocal, DVE multiply.  Head 0
  writes O^T directly; head 1 goes through one partition-shifting
  SBUF->SBUF DMA.
"""

import numpy as np

P = 128
B, S, D = 2, 2048, 1024
H, DK = 16, 64
COLS = 256          # qkv columns per core (4 heads)
KC = D // P         # 8 contraction chunks for the projections
TT = 512            # token block (matmul free dim)
NJ = S // TT        # 4 token blocks
NT = S // P         # 16 token tiles
NKT = S // P        # 16 key tiles
VW = 65             # per-head AV lhsT width: 64 v-dims + ones column

_CACHE = {}


def _build():
    import concourse.bass as bass
    import concourse.tile as tile
    from concourse import bacc, mybir

    f32 = mybir.dt.float32
    bf16 = mybir.dt.bfloat16
    Exp = mybir.ActivationFunctionType.Exp

    nc = bacc.Bacc(
        "TRN2", target_bir_lowering=False, debug=False,
        enable_asserts=False, num_devices=8,
    )
    # all inputs ship host-pre-arranged so every partition's DMA line is
    # one contiguous 4-8KB descriptor instead of 512B strips (the DMA
    # fabric is descriptor-bound: 0.5MB of 512B descriptors took ~5.5us)
    xt_d = nc.dram_tensor("xt", [P, NJ, KC, TT], bf16, kind="ExternalInput").ap()
    wq_d = nc.dram_tensor("wq", [P, KC, COLS], bf16, kind="ExternalInput").ap()
    wk_d = nc.dram_tensor("wk", [P, KC, COLS], bf16, kind="ExternalInput").ap()
    wv_d = nc.dram_tensor("wv", [P, KC, COLS], bf16, kind="ExternalInput").ap()
    wo_d = nc.dram_tensor("wo", [P, 2, D], bf16, kind="ExternalInput").ap()
    bq_d = nc.dram_tensor("bq", [P, 2], f32, kind="ExternalInput").ap()
    bk_d = nc.dram_tensor("bk", [P, 2], f32, kind="ExternalInput").ap()
    out_d = nc.dram_tensor("out_t", [D, S], bf16, kind="ExternalOutput").ap()

    with tile.TileContext(nc) as tc:
        with (
            tc.tile_pool(name="const", bufs=1) as const,
            tc.tile_pool(name="wpool", bufs=1) as wpool,
            tc.tile_pool(name="persist", bufs=1) as persist,
            tc.tile_pool(name="exps", bufs=3) as exps,
            tc.tile_pool(name="stage", bufs=4) as stage,
            tc.tile_pool(name="outst", bufs=4) as outst,
            tc.tile_pool(name="ps_sc", bufs=1, space="PSUM") as ps_sc,
            tc.tile_pool(name="ps_acc", bufs=1, space="PSUM") as ps_acc,
            tc.tile_pool(name="ps_u", bufs=2, space="PSUM") as ps_u,
        ):
            ones_b = const.tile([P, VW], bf16, tag="ones_b")
            nc.vector.memset(ones_b[:], 1.0)

            # warm the PE p-state (0.65 -> 2.4GHz ramps after ~3us of
            # continuous execution) with throwaway matmuls during the
            # otherwise-idle windows while the weight/x DMAs land; the
            # lead-in chains then run at full rate instead of ~390-630ns/mm
            dummy = const.tile([P, TT], bf16, tag="dummy")
            nc.vector.memset(dummy[:], 0.0)
            warm_box = {}

            def warm(n, fresh=False):
                if fresh or "t" not in warm_box:
                    warm_box["t"] = ps_u.tile([P, TT], f32, tag="u",
                                              name="warm")
                for _ in range(n):
                    nc.tensor.matmul(
                        warm_box["t"][0:VW, :], ones_b[:, :], dummy[:, :],
                        start=True, stop=True,
                    )

            warm(20)

            # ---- weights: already bf16 in DRAM (host-side cast); issued on
            # the scalar queue so they don't serialize behind the x DMAs on
            # the sync queue.  Biases come pre-shaped [128, 2] from the host
            # (a "(o p) -> p o" DRAM gather is 256 4-byte descriptors that
            # crawl through the DMA fabric) and load first — the first
            # K bias-add otherwise stalls the whole projection chain. ----
            # split wk: the first two chunks land fast so the very first
            # projection chain isn't gated on the full 0.5MB transfer
            wk_r0 = wpool.tile([P, 2, COLS], bf16, tag="w_k0")
            nc.scalar.dma_start(wk_r0[:], wk_d[:, 0:2, :])
            # wk_r1 rides the sync queue ahead of the x tiles so it lands
            # in parallel with wk_r0 instead of serializing behind it
            wk_r1 = wpool.tile([P, KC - 2, COLS], bf16, tag="w_k1")
            nc.sync.dma_start(wk_r1[:], wk_d[:, 2:, :])
            bq_sb = const.tile([P, 2], f32, tag="bq")
            nc.scalar.dma_start(bq_sb[:], bq_d)
            bk_sb = const.tile([P, 2], f32, tag="bk")
            nc.scalar.dma_start(bk_sb[:], bk_d)
            wv_r = wpool.tile([P, KC, COLS], bf16, tag="w_v")
            nc.scalar.dma_start(wv_r[:], wv_d)
            wq_r = wpool.tile([P, KC, COLS], bf16, tag="w_q")
            nc.scalar.dma_start(wq_r[:], wq_d)
            wo_r = wpool.tile([P, 2, D], bf16, tag="w_o")
            nc.scalar.dma_start(wo_r[:], wo_d)

            # persistent activations (all bf16)
            qT = persist.tile([P, 2, S], bf16, tag="qT")    # [qcol, tok]
            kT = persist.tile([P, 2, S], bf16, tag="kT")    # [kcol, tok]
            vt = persist.tile([P, NT, 4 * VW], bf16, tag="vt")  # [tok, h*(V|1)]
            oT = persist.tile([P, 2, S], bf16, tag="oT")    # [vdim, tok]
            xTs = [persist.tile([P, KC, TT], bf16, tag=f"xT{j}", name=f"xT{j}")
                   for j in range(NJ)]

            # ones column (index 64 of each head's VW slice)
            vt_heads = vt[:].rearrange("p t (h c) -> p t h c", c=VW)
            nc.vector.tensor_copy(
                vt_heads[:, :, :, 64],
                ones_b[:, :NT * 4].rearrange("p (t h) -> p t h", h=4),
            )

            for j in range(NJ):
                nc.sync.dma_start(xTs[j][:], xt_d[:, j])

            # ---- projection emitters ----
            def wk_sel(kc):
                return wk_r0[:, kc, :] if kc < 2 else wk_r1[:, kc - 2, :]

            def wq_sel(kc):
                return wq_r[:, kc, :]

            def qk_proj_ct(j, wsel, bsb, dstT, ct, acc, kc0, kc1):
                for kc in range(kc0, kc1):
                    nc.tensor.matmul(
                        acc[:], wsel(kc)[:, bass.ts(ct, P)], xTs[j][:, kc, :],
                        start=(kc == 0), stop=(kc == KC - 1),
                    )
                if kc1 == KC:
                    nc.vector.tensor_scalar_add(
                        dstT[:, ct, bass.ts(j, TT)], acc[:], bsb[:, ct : ct + 1]
                    )

            def v_proj(j, ts_):
                acc = ps_u.tile([P, COLS], f32, tag="u", name="v_acc")
                for kc in range(KC):
                    nc.tensor.matmul(
                        acc[:], xTs[j][:, kc, bass.ts(ts_, P)], wv_r[:, kc, :],
                        start=(kc == 0), stop=(kc == KC - 1),
                    )
                tt = 4 * j + ts_
                nc.vector.tensor_copy(
                    vt_heads[:, tt, :, 0:64],
                    acc[:].rearrange("p (h c) -> p h c", c=64),
                )

            # ---- phase A (lead-in): only what block (0,0) needs up front:
            # K(0) ct0, V(0), Q(0) ct0.  Everything else becomes PE filler
            # work inside the ACT-paced attention loop — PE is the global
            # bottleneck, so projection work must hide under the exp
            # stream instead of serializing before it. ----
            acc = ps_u.tile([P, TT], f32, tag="u", name="k_acc")
            qk_proj_ct(0, wk_sel, bk_sb, kT, 0, acc, 0, 2)
            warm(12)   # hold the p-state through the wk_r1 arrival window
            qk_proj_ct(0, wk_sel, bk_sb, kT, 0, acc, 2, KC)
            for ts_ in range(TT // P):
                v_proj(0, ts_)
            acc = ps_u.tile([P, TT], f32, tag="u", name="q_acc")
            qk_proj_ct(0, wq_sel, bq_sb, qT, 0, acc, 0, KC)

            # ---- filler queue: atomic groups of PE work (projection column
            # tiles, V units, out-projection units) drained a few steps per
            # kc slot inside the attention loop.  A multi-slot group is only
            # started when it fits in the current block's remaining slots,
            # so a ps_u accumulation never straddles the block boundary
            # where the norm's rbc tiles rotate through ps_u (that
            # interleaving could deadlock the in-order PE queue). ----
            fillers = []      # list of (key, [step closures])
            active = []       # remaining steps of the started group
            active_key = [None]
            done_keys = set()
            # produced in the lead-in:
            done_keys.update([("k", 0, 0), ("q", 0, 0)])
            done_keys.update([("v", 0, ts_) for ts_ in range(4)])

            def qkproj_group(j, ct, wsel, bsb, dstT, nm):
                box = {}
                def step(kc0, box=box):
                    if kc0 == 0:
                        box["acc"] = ps_u.tile([P, TT], f32, tag="u", name=nm)
                    qk_proj_ct(j, wsel, bsb, dstT, ct, box["acc"],
                               kc0, kc0 + 1)
                return [lambda kc0=kc0: step(kc0) for kc0 in range(KC)]

            def vproj_group(j, ts_):
                box = {}
                def step(kc0, box=box):
                    if kc0 == 0:
                        box["acc"] = ps_u.tile([P, COLS], f32, tag="u",
                                               name="v_acc")
                    acc = box["acc"]
                    for kc in range(kc0, kc0 + 2):
                        nc.tensor.matmul(
                            acc[:], xTs[j][:, kc, bass.ts(ts_, P)],
                            wv_r[:, kc, :],
                            start=(kc == 0), stop=(kc == KC - 1),
                        )
                    if kc0 == KC - 2:
                        tt = 4 * j + ts_
                        nc.vector.tensor_copy(
                            vt_heads[:, tt, :, 0:64],
                            acc[:].rearrange("p (h c) -> p h c", c=64),
                        )
                return [lambda kc0=kc0: step(kc0) for kc0 in range(0, KC, 2)]

            tail_acc_box = {}

            def outproj_group(j, oc):
                box = {}

                def s1():
                    if j == NJ - 1 and oc % 4 >= 2:
                        # tail units: the attention o_ps banks are free
                        # after the final norm copies — borrow them so the
                        # PSUM rotation is 4-deep instead of 2-deep
                        if oc % 4 == 2:
                            tail_acc_box["t"] = ps_acc.tile(
                                [P, 2, TT], f32, tag="acc", name="tail_acc"
                            )
                        box["acc"] = tail_acc_box["t"][:, oc % 2, :]
                    else:
                        box["acc"] = ps_u.tile([P, TT], f32, tag="u",
                                               name="wo_acc")[:]
                    nc.tensor.matmul(
                        box["acc"], wo_r[:, 0, bass.ts(oc, P)],
                        oT[:, 0, bass.ts(j, TT)], start=True, stop=False,
                    )

                def s2():
                    acc = box["acc"]
                    nc.tensor.matmul(
                        acc, wo_r[:, 1, bass.ts(oc, P)],
                        oT[:, 1, bass.ts(j, TT)], start=False, stop=True,
                    )
                    st = outst.tile([P, TT], bf16, tag="outst", name="outst")
                    if j == NJ - 1 and oc % 2 == 0:
                        # the last j's units drain after the final exp:
                        # alternate the then-idle ACT engine with DVE so the
                        # tail's PSUM->SBUF copies run on two engines
                        nc.scalar.copy(st[:], acc)
                    else:
                        nc.vector.tensor_copy(st[:], acc)
                    nc.sync.dma_start(
                        out_d[bass.ts(oc, P), bass.ts(j, TT)], st[:]
                    )

                return [s1, s2]

            def _finish_active():
                while active:
                    active.pop(0)()
                if active_key[0] is not None:
                    done_keys.add(active_key[0])
                    active_key[0] = None

            def drain_filler(slots_left, n=1):
                for _ in range(n):
                    if not active:
                        if active_key[0] is not None:
                            done_keys.add(active_key[0])
                            active_key[0] = None
                        for gi, (key, grp) in enumerate(fillers):
                            if len(grp) <= slots_left:
                                key, grp = fillers.pop(gi)
                                active.extend(grp)
                                active_key[0] = key
                                break
                        else:
                            return
                    active.pop(0)()
                if not active and active_key[0] is not None:
                    done_keys.add(active_key[0])
                    active_key[0] = None

            def ensure(key):
                # force-emit producer groups (in queue order) until `key`
                # has been fully emitted.  Called before the consumer is
                # emitted so the dependency is recorded.
                if key in done_keys:
                    return
                if active_key[0] == key:
                    _finish_active()
                    return
                while key not in done_keys:
                    _finish_active()
                    if not fillers:
                        raise RuntimeError(f"missing producer {key}")
                    k, grp = fillers.pop(0)
                    active.extend(grp)
                    active_key[0] = k
                _finish_active()

            # production order: per j, the K/Q ct0 and V needed by the p=0
            # blocks; then all ct1 work needed by the p=1 blocks.
            for j in range(1, NJ):
                fillers.append((("k", j, 0),
                                qkproj_group(j, 0, wk_sel, bk_sb, kT, "k_acc")))
                fillers.append((("q", j, 0),
                                qkproj_group(j, 0, wq_sel, bq_sb, qT, "q_acc")))
                for ts_ in range(TT // P):
                    fillers.append((("v", j, ts_), vproj_group(j, ts_)))
            for j in range(NJ):
                fillers.append((("k", j, 1),
                                qkproj_group(j, 1, wk_sel, bk_sb, kT, "k_acc")))
            for j in range(NJ):
                fillers.append((("q", j, 1),
                                qkproj_group(j, 1, wq_sel, bq_sb, qT, "q_acc")))

            # ---- normalization, split in two parts: the PSUM->SBUF copies
            # (the only o_ps reads) are emitted right after the last AV so
            # the next block's o_ps alloc records them; the arithmetic runs
            # after the next block's first scores so PE/ACT keep flowing ----
            def norm_copies(o_ps):
                osbs = []
                for i in range(2):
                    osb = stage.tile([VW, TT], bf16, tag="osb", name="osb")
                    nc.vector.tensor_copy(osb[:], o_ps[0:VW, i, :])
                    osbs.append(osb)
                return osbs

            def norm_arith(j, p, osbs):
                for i in range(2):
                    osb = osbs[i]
                    rbc = ps_u.tile([64, TT], f32, tag="u", name="rbc")
                    nc.tensor.matmul(
                        rbc[:], ones_b[64:65, 0:64], osb[64:65, :],
                        start=True, stop=True,
                    )
                    rbs = stage.tile([64, TT], f32, tag="rbs", name="rbs")
                    nc.vector.reciprocal_approx_fast(rbs[:], rbc[:])
                    if i == 0:
                        nc.vector.tensor_tensor(
                            oT[0:64, p, bass.ts(j, TT)], osb[0:64, :], rbs[:],
                            mybir.AluOpType.mult,
                        )
                    else:
                        onrm = stage.tile([64, TT], bf16, tag="onrm",
                                          name="onrm")
                        nc.vector.tensor_tensor(
                            onrm[:], osb[0:64, :], rbs[:], mybir.AluOpType.mult
                        )
                        nc.sync.dma_start(
                            oT[64:128, p, bass.ts(j, TT)], onrm[:]
                        )

            # scores PSUM: two parity tiles of 2 banks each.  Separate tiles
            # (not one [P,4,TT] tensor) so the tile-granular WAR dependency
            # lets sc(kc+2) overlap exp(kc+1): one tile would serialize every
            # score matmul behind the latest exp read, collapsing the
            # pipeline to 1-deep (measured 1.66us/kc vs ACT's 1.11us).
            big_scs = [ps_sc.tile([P, 2, TT], f32, tag=f"sc{par}",
                                  name=f"sc{par}") for par in range(2)]

            # ---- attention: ACT-paced kc pipeline with PE fillers.  The
            # next block's first two score pairs are emitted during the
            # current block's last two kc slots so the exp stream crosses
            # block boundaries without a bubble. ----
            blocks = [(j, p) for p in range(2) for j in range(NJ)]

            def sc_emit_b(t, kc):
                j, p = blocks[t]
                ensure(("k", kc // 4, p))
                ensure(("q", j, p))
                sc = big_scs[kc % 2]
                for i in range(2):
                    lo, hi = 64 * i, 64 * i + 64
                    nc.tensor.matmul(
                        sc[:, i, :],
                        kT[lo:hi, p, bass.ts(kc, P)],
                        qT[lo:hi, p, bass.ts(j, TT)],
                        start=True, stop=True,
                    )

            pending_norm = None
            for t, (j, p) in enumerate(blocks):
                o_ps = ps_acc.tile([P, 2, TT], f32, tag="acc", name="o_ps")

                def av_emit(kc, ex, p=p, o_ps=o_ps):
                    ensure(("v", kc // 4, kc % 4))
                    for i in range(2):
                        h = 2 * p + i
                        nc.tensor.matmul(
                            o_ps[0:VW, i, :],
                            vt[:, kc, bass.ds(VW * h, VW)],
                            ex[:, i, :],
                            start=(kc == 0), stop=(kc == NKT - 1),
                        )

                if t == 0:
                    sc_emit_b(0, 0)
                    sc_emit_b(0, 1)
                if pending_norm is not None:
                    pending_norm()
                    pending_norm = None
                # block 0 must mass-produce K/V for its own kc stream; later
                # p=0 blocks only need their own Q/K ct0 (force-ensured), so
                # defer the rest into the ACT-paced slack of the p=1 blocks
                ndrain = 3 if t == 0 else 1
                prev = None
                for kc in range(NKT):
                    ex = exps.tile([P, 2, TT], bf16, tag="exp", name="ex")
                    nc.scalar.activation(
                        ex[:], big_scs[kc % 2][:], Exp, scale=0.125,
                    )
                    drain_filler(NKT - kc, ndrain)
                    if prev is not None:
                        av_emit(kc - 1, prev)
                    if kc + 2 < NKT:
                        sc_emit_b(t, kc + 2)
                    elif t + 1 < len(blocks):
                        sc_emit_b(t + 1, kc - (NKT - 2))
                    prev = ex
                av_emit(NKT - 1, prev)
                osbs = norm_copies(o_ps)
                pending_norm = (
                    lambda j=j, p=p, osbs=osbs: norm_arith(j, p, osbs)
                )
                if p == 1:
                    for oc in range(D // P):
                        fillers.append((("o", j, oc), outproj_group(j, oc)))
            pending_norm()
            # keep the PE p-state up through the final norm's DVE chain so
            # the tail out-projection matmuls run warm (measured 634ns/mm
            # after a ~2.5us PE gap here, vs 241ns warm)
            warm(8, fresh=True)
            while fillers or active:
                drain_filler(NKT)

    nc.compile()
    return nc


def make_in_maps(x, Wq, bq, Wk, bk, Wv, Wo):
    import ml_dtypes

    bf = ml_dtypes.bfloat16

    def arr_w(w, rows):
        # [rows*P, F] -> [P, rows, F], d = row*P + p
        return np.ascontiguousarray(
            w.astype(bf).reshape(rows, P, -1).transpose(1, 0, 2))

    xt = [np.ascontiguousarray(
        x[b].T.astype(bf).reshape(KC, P, NJ, TT).transpose(1, 2, 0, 3))
        for b in range(B)]

    in_maps = []
    for c in range(8):
        b, g = divmod(c, 4)
        cs = slice(COLS * g, COLS * (g + 1))
        in_maps.append({
            "xt": xt[b],
            "wq": arr_w(Wq[:, cs], KC),
            "wk": arr_w(Wk[:, cs], KC),
            "wv": arr_w(Wv[:, cs], KC),
            "wo": arr_w(Wo[cs, :], 2),
            "bq": np.ascontiguousarray(bq[cs].reshape(2, P).T),
            "bk": np.ascontiguousarray(bk[cs].reshape(2, P).T),
        })
    return in_maps


def kernel(x, Wq, bq, Wk, bk, Wv, bv, Wo, bo):
    from concourse import bass_utils

    x = np.asarray(x, dtype=np.float32)
    Wq = np.asarray(Wq, dtype=np.float32)
    Wk = np.asarray(Wk, dtype=np.float32)
    Wv = np.asarray(Wv, dtype=np.float32)
    Wo = np.asarray(Wo, dtype=np.float32)
    bq = np.asarray(bq, dtype=np.float32)
    bk = np.asarray(bk, dtype=np.float32)
    bv = np.asarray(bv, dtype=np.float32)
    bo = np.asarray(bo, dtype=np.float32)

    if "nc" not in _CACHE:
        _CACHE["nc"] = _build()
    nc = _CACHE["nc"]

    in_maps = make_in_maps(x, Wq, bq, Wk, bk, Wv, Wo)
    res = bass_utils.run_bass_kernel_spmd(nc, in_maps, core_ids=list(range(8)))

    out = np.zeros((B, S, D), dtype=np.float32)
    for c in range(8):
        out[c // 4] += res.results[c]["out_t"].T.astype(np.float32)
    out += bo + bv @ Wo
    return out
